# revision 10
# baseline (speedup 1.0000x reference)
"""Trainium2 Bass kernel for nn_LocalFmoeCatEmbedFeedForward.

Strategy (expert-parallel, 8 cores):
  - Host: router (concat -> logits -> softmax -> top-1 gate) + dispatch.
    Tokens are gathered per expert; each expert's tokens split across 2
    cores (4 experts x 2 = 8 cores). Gate is applied host-side to the
    OUTPUT (y_final = gate * (y_dev + b2)), so the device program needs
    no per-token scaling at all.
  - Device (per core), all matmuls in bf16 (same 1 cyc/row PE rate as
    fp32r but half the DMA bytes — the baseline was DMA-limited at the
    head/tail):
      GEMM1: hT[m, t] = relu(sum_k W1T[k,m].T @ xT[k, t] + b1[m])
      GEMM2: yT[d, t] = sum_k W2T[k,d].T @ hT[k, t]
    Both keep tokens on the free dim, so shard sizes need no 128
    rounding, and GEMM1's output layout directly feeds GEMM2's moving
    operand (no transposes anywhere).
  - All x/h/y buffers are chunk-major ([p][chunk][k][tok]) so every DMA
    is one fully-contiguous block with 4KB-per-partition runs — the DMA
    engines are descriptor-rate-bound, so run length sets bandwidth.
  - Host: scatter rows back, add b2 if nonzero, scale by gate.
"""

import os
import sys

sys.path.insert(0, "/opt/trn_rl_repo")

import numpy as np
import ml_dtypes

import concourse.bacc as bacc
import concourse.tile as tile
from concourse import mybir
from concourse import bass_utils

IDIM, EMBED_DIM, NUM_EXPERTS, HIDDEN = 512, 256, 4, 1024
N_CORES = 8
P = 128
K1 = IDIM // P     # 4   k-blocks for GEMM1
M1 = HIDDEN // P   # 8   m-blocks (h features) = GEMM2's k-blocks
K2 = HIDDEN // P   # 8
D1 = IDIM // P     # 4   d-blocks (output features)

BF16 = mybir.dt.bfloat16
NPBF16 = ml_dtypes.bfloat16


def _chunks_for(C: int):
    """256-token first chunk (early PE start while DMA streams), 512
    steady-state, small final chunk (short drain tail)."""
    chunks = []
    n0 = 0
    if C > 256:
        chunks.append((0, 256))
        n0 = 256
    while C - n0 >= 512 + 128:
        chunks.append((n0, 512))
        n0 += 512
    rem = C - n0
    if rem > 128:
        chunks.append((n0, rem - 128))
        n0 += rem - 128
    if C - n0 > 0:
        chunks.append((n0, C - n0))
    return chunks


def _build_nc(C: int):
    """Per-core SPMD program for a token capacity of C (multiple of 32)."""
    nc = bacc.Bacc("TRN2", target_bir_lowering=False, debug=False,
                   num_devices=N_CORES)
    f32 = mybir.dt.float32

    xT = nc.dram_tensor("xT", [P, K1 * C], BF16, kind="ExternalInput").ap()
    w1p = nc.dram_tensor("w1p", [P, M1 * K1 * P], BF16, kind="ExternalInput").ap()
    w2p = nc.dram_tensor("w2p", [P, K2 * D1 * P], BF16, kind="ExternalInput").ap()
    b1 = nc.dram_tensor("b1", [P, M1], f32, kind="ExternalInput").ap()
    y = nc.dram_tensor("y", [P, D1 * C], BF16, kind="ExternalOutput").ap()

    chunks = _chunks_for(C)
    NCH = len(chunks)

    with tile.TileContext(nc) as tc:
        with (
            tc.tile_pool(name="xt", bufs=1) as xt_pool,
            tc.tile_pool(name="w", bufs=1) as w_pool,
            tc.tile_pool(name="ht", bufs=1) as ht_pool,
            tc.tile_pool(name="sm", bufs=1) as sm_pool,
            tc.tile_pool(name="yo", bufs=4) as yo_pool,
            tc.tile_pool(name="ps1", bufs=4, space="PSUM") as ps1_pool,
            tc.tile_pool(name="ps2", bufs=4, space="PSUM") as ps2_pool,
        ):
            w1a = w_pool.tile([P, M1 * K1 * P], BF16, tag="w1a", name="w1a")
            xt = xt_pool.tile([P, K1 * C], BF16, tag="xt", name="xt")

            def load_chunk(ci):
                n0, w = chunks[ci]
                nc.sync.dma_start(xt[:, K1 * n0:K1 * (n0 + w)],
                                  xT[:, K1 * n0:K1 * (n0 + w)])

            def load_w1(m0, m1):
                nc.sync.dma_start(w1a[:, m0 * K1 * P:m1 * K1 * P],
                                  w1p[:, m0 * K1 * P:m1 * K1 * P])

            # PE p-state warm-up: throwaway matmuls on a memset tile while
            # the input DMAs stream, so real matmuls start at full clock.
            wu = sm_pool.tile([P, 512], BF16, tag="wu")
            nc.vector.memset(wu[:], 0.0)
            for r in range(12):
                psw = ps2_pool.tile([P, 512], f32, tag="ps2")
                nc.tensor.matmul(psw[:], wu[:, 0:P], wu[:], start=True,
                                 stop=True)

            # Head: interleave w1 m-blocks with x chunks so GEMM1's m-loop
            # on chunk0 is never starved; small tensors go via the
            # Activation queue to keep the Sync issue stream tight.
            b1_sb = sm_pool.tile([P, M1], f32, tag="b1")
            nc.scalar.dma_start(b1_sb[:], b1[:])
            load_w1(0, 1)
            load_chunk(0)
            load_w1(1, 3)
            if NCH > 1:
                load_chunk(1)
            load_w1(3, 6)
            if NCH > 2:
                load_chunk(2)
            load_w1(6, M1)
            w2a = w_pool.tile([P, K2 * D1 * P], BF16, tag="w2a", name="w2a")
            if NCH > 3:
                load_chunk(3)
            nc.sync.dma_start(w2a[:, 0:K2 * D1 * P // 2],
                              w2p[:, 0:K2 * D1 * P // 2])
            if NCH > 4:
                load_chunk(4)
            nc.sync.dma_start(w2a[:, K2 * D1 * P // 2:],
                              w2p[:, K2 * D1 * P // 2:])
            for ci in range(5, NCH):
                load_chunk(ci)

            ht = ht_pool.tile([P, M1 * C], BF16, tag="ht", name="ht")

            def g1(ci):
                n0, w = chunks[ci]
                cb = K1 * n0
                hb = M1 * n0
                for m in range(M1):
                    ps = ps1_pool.tile([P, 512], f32, tag="ps1")
                    for k in range(K1):
                        nc.tensor.matmul(
                            ps[:, :w],
                            w1a[:, (m * K1 + k) * P:(m * K1 + k + 1) * P],
                            xt[:, cb + k * w:cb + (k + 1) * w],
                            start=(k == 0),
                            stop=(k == K1 - 1),
                        )
                    nc.scalar.activation(
                        ht[:, hb + m * w:hb + (m + 1) * w], ps[:, :w],
                        mybir.ActivationFunctionType.Relu,
                        bias=b1_sb[:, m:m + 1],
                    )

            def g2(ci):
                n0, w = chunks[ci]
                hb = M1 * n0
                last = ci == NCH - 1
                yt = yo_pool.tile([P, D1 * 512], BF16, tag="yo")
                for d in range(D1):
                    ps = ps2_pool.tile([P, 512], f32, tag="ps2")
                    for k in range(K2):
                        nc.tensor.matmul(
                            ps[:, :w],
                            w2a[:, (k * D1 + d) * P:(k * D1 + d + 1) * P],
                            ht[:, hb + k * w:hb + (k + 1) * w],
                            start=(k == 0),
                            stop=(k == K2 - 1),
                        )
                    # psum -> sbuf cast on the otherwise-idle Vector engine
                    nc.vector.tensor_copy(yt[:, d * w:(d + 1) * w], ps[:, :w])
                    if last:
                        # drain the final chunk per d-block so the last
                        # transfer overlaps the remaining matmuls
                        nc.scalar.dma_start(
                            y[:, D1 * n0 + d * w:D1 * n0 + (d + 1) * w],
                            yt[:, d * w:(d + 1) * w])
                if not last:
                    # One contiguous DMA per chunk otherwise.
                    nc.scalar.dma_start(y[:, D1 * n0:D1 * (n0 + w)],
                                        yt[:, 0:D1 * w])

            # Software pipeline: GEMM2 of chunk i runs one chunk behind
            # GEMM1 so the ReLU activations have time to drain.
            g1(0)
            for ci in range(1, NCH):
                g1(ci)
                g2(ci - 1)
            g2(NCH - 1)

    nc.compile()
    return nc


def _pack_chunk_major(mat, chunks, kb):
    """[C, kb*P] row-major -> [P, kb*C] chunk-major ([p][chunk][k][tok])."""
    segs = []
    for n0, w in chunks:
        seg = mat[n0:n0 + w].T.reshape(kb, P, w).transpose(1, 0, 2)
        segs.append(seg.reshape(P, kb * w))
    return np.ascontiguousarray(np.concatenate(segs, axis=1))


def kernel(inputs, embed, router_weights, w1_weight, w1_bias, w2_weight,
           w2_bias, mask):
    inputs = np.asarray(inputs, np.float32)
    embed = np.asarray(embed, np.float32)
    router_weights = np.asarray(router_weights, np.float32)
    w1_weight = np.asarray(w1_weight, np.float32)
    w1_bias = np.asarray(w1_bias, np.float32)
    w2_weight = np.asarray(w2_weight, np.float32)
    w2_bias = np.asarray(w2_bias, np.float32)
    mask_f = np.asarray(mask).astype(np.float32)

    B, T, D = inputs.shape
    N = B * T
    x = inputs.reshape(N, D)

    # ---- host router: softmax top-1 over concat(embed, inputs) ----
    router_in = np.concatenate([embed.reshape(N, EMBED_DIM), x], axis=1)
    logits = router_in @ router_weights
    logits -= logits.max(axis=1, keepdims=True)
    p = np.exp(logits)
    p /= p.sum(axis=1, keepdims=True)
    gate_idx = np.argmax(p, axis=1)
    gate_val = p[np.arange(N), gate_idx] * mask_f.reshape(N)

    # ---- dispatch: expert e -> cores 2e, 2e+1 ----
    shard_idx = []
    for e in range(NUM_EXPERTS):
        te = np.nonzero(gate_idx == e)[0]
        h = (len(te) + 1) // 2
        shard_idx.append(te[:h])
        shard_idx.append(te[h:])
    C = max(32, -(-max(len(s) for s in shard_idx) // 32) * 32)

    nc = _build_nc(C)
    chunks = _chunks_for(C)

    in_maps = []
    for c in range(N_CORES):
        e = c // 2
        idx = shard_idx[c]
        xs = np.zeros((C, D), np.float32)
        xs[: len(idx)] = x[idx]
        xTp = _pack_chunk_major(xs, chunks, K1).astype(NPBF16)
        # w1p [P, M1*K1*P]: [p, m, k, j] = W1T[k*128+p, m*128+j]
        w1p = np.ascontiguousarray(
            w1_weight[e].T.reshape(K1, P, M1, P)
            .transpose(1, 2, 0, 3).reshape(P, M1 * K1 * P)
        ).astype(NPBF16)
        # w2p [P, K2*D1*P]: [p, k, d, j] = W2T[k*128+p, d*128+j]
        w2p = np.ascontiguousarray(
            w2_weight[e].T.reshape(K2, P, D1, P)
            .transpose(1, 0, 2, 3).reshape(P, K2 * D1 * P)
        ).astype(NPBF16)
        b1p = np.ascontiguousarray(w1_bias[e].reshape(M1, P).T)
        in_maps.append({"xT": xTp, "w1p": w1p, "w2p": w2p, "b1": b1p})

    trace = bool(os.environ.get("KERNEL_TRACE"))
    kw = {}
    if trace:
        bass_utils.upload_artifacts = lambda tmpdir: f"local:{tmpdir}"
        kw = dict(trace=True, trace_cores=list(range(N_CORES)),
                  tmpdir=os.environ.get("KERNEL_TRACE_DIR") or None)
    try:
        res = bass_utils.run_bass_kernel_spmd(
            nc, in_maps, core_ids=list(range(N_CORES)), **kw)
    except Exception:
        res = bass_utils.run_bass_kernel_spmd(
            nc, in_maps, core_ids=list(range(N_CORES)), **kw)
    if trace:
        kernel.exec_time_ns = res.exec_time_ns
        kernel.mean_exec_time_ns = res.mean_exec_time_ns

    out = np.zeros((N, D), np.float32)
    for c in range(N_CORES):
        idx = shard_idx[c]
        arr = np.asarray(res.results[c]["y"]).astype(np.float32)
        rows = np.empty((C, D), np.float32)
        for n0, w in chunks:
            seg = arr[:, D1 * n0:D1 * (n0 + w)].reshape(P, D1, w)
            rows[n0:n0 + w] = seg.transpose(2, 1, 0).reshape(w, D1 * P)
        out[idx] = rows[: len(idx)]
    if np.any(w2_bias):
        out += w2_bias[gate_idx]
    out *= gate_val[:, None]
    return out.reshape(B, T, D)


# revision 12
# speedup vs baseline: 1.0458x; 1.0458x over previous
"""Trainium2 Bass kernel for nn_LocalFmoeCatEmbedFeedForward.

Strategy (expert-parallel, 8 cores):
  - Host: router (concat -> logits -> softmax -> top-1 gate) + dispatch.
    Tokens are gathered per expert; each expert's tokens split across 2
    cores (4 experts x 2 = 8 cores). Gate is applied host-side to the
    OUTPUT (y_final = gate * (y_dev + b2)), so the device program needs
    no per-token scaling at all.
  - Device (per core), all matmuls in bf16 (same 1 cyc/row PE rate as
    fp32r but half the DMA bytes — the baseline was DMA-limited at the
    head/tail):
      GEMM1: hT[m, t] = relu(sum_k W1T[k,m].T @ xT[k, t] + b1[m])
      GEMM2: yT[d, t] = sum_k W2T[k,d].T @ hT[k, t]
    Both keep tokens on the free dim, so shard sizes need no 128
    rounding, and GEMM1's output layout directly feeds GEMM2's moving
    operand (no transposes anywhere).
  - All x/h/y buffers are chunk-major ([p][chunk][k][tok]) so every DMA
    is one fully-contiguous block with 4KB-per-partition runs — the DMA
    engines are descriptor-rate-bound, so run length sets bandwidth.
  - Host: scatter rows back, add b2 if nonzero, scale by gate.
"""

import os
import sys

sys.path.insert(0, "/opt/trn_rl_repo")

import numpy as np
import ml_dtypes

import concourse.bacc as bacc
import concourse.tile as tile
from concourse import mybir
from concourse import bass_utils

IDIM, EMBED_DIM, NUM_EXPERTS, HIDDEN = 512, 256, 4, 1024
N_CORES = 8
P = 128
K1 = IDIM // P     # 4   k-blocks for GEMM1
M1 = HIDDEN // P   # 8   m-blocks (h features) = GEMM2's k-blocks
K2 = HIDDEN // P   # 8
D1 = IDIM // P     # 4   d-blocks (output features)

BF16 = mybir.dt.bfloat16
NPBF16 = ml_dtypes.bfloat16


def _chunks_for(C: int):
    """256-token first chunk (early PE start while DMA streams), 512
    steady-state, small final chunk (short drain tail)."""
    chunks = []
    n0 = 0
    if C > 256:
        chunks.append((0, 256))
        n0 = 256
    while C - n0 >= 512 + 128:
        chunks.append((n0, 512))
        n0 += 512
    rem = C - n0
    if rem > 128:
        chunks.append((n0, rem - 128))
        n0 += rem - 128
    if C - n0 > 0:
        chunks.append((n0, C - n0))
    return chunks


def _build_nc(C: int):
    """Per-core SPMD program for a token capacity of C (multiple of 32)."""
    nc = bacc.Bacc("TRN2", target_bir_lowering=False, debug=False,
                   num_devices=N_CORES)
    f32 = mybir.dt.float32

    xT = nc.dram_tensor("xT", [P, K1 * C], BF16, kind="ExternalInput").ap()
    w1p = nc.dram_tensor("w1p", [P, M1 * K1 * P], BF16, kind="ExternalInput").ap()
    w2p = nc.dram_tensor("w2p", [P, K2 * D1 * P], BF16, kind="ExternalInput").ap()
    b1 = nc.dram_tensor("b1", [P, M1], f32, kind="ExternalInput").ap()
    y = nc.dram_tensor("y", [P, D1 * C], BF16, kind="ExternalOutput").ap()

    chunks = _chunks_for(C)
    NCH = len(chunks)

    with tile.TileContext(nc) as tc:
        with (
            tc.tile_pool(name="xt", bufs=1) as xt_pool,
            tc.tile_pool(name="w", bufs=1) as w_pool,
            tc.tile_pool(name="ht", bufs=1) as ht_pool,
            tc.tile_pool(name="sm", bufs=1) as sm_pool,
            tc.tile_pool(name="yo", bufs=4) as yo_pool,
            tc.tile_pool(name="ps1", bufs=4, space="PSUM") as ps1_pool,
            tc.tile_pool(name="ps2", bufs=4, space="PSUM") as ps2_pool,
        ):
            w1a = w_pool.tile([P, M1 * K1 * P], BF16, tag="w1a", name="w1a")
            xt = xt_pool.tile([P, K1 * C], BF16, tag="xt", name="xt")

            def load_chunk(ci):
                n0, w = chunks[ci]
                nc.sync.dma_start(xt[:, K1 * n0:K1 * (n0 + w)],
                                  xT[:, K1 * n0:K1 * (n0 + w)])

            def load_w1(m0, m1):
                nc.sync.dma_start(w1a[:, m0 * K1 * P:m1 * K1 * P],
                                  w1p[:, m0 * K1 * P:m1 * K1 * P])

            # PE p-state warm-up: throwaway matmuls on a memset tile while
            # the input DMAs stream, so real matmuls start at full clock.
            wu = sm_pool.tile([P, 512], BF16, tag="wu")
            nc.vector.memset(wu[:], 0.0)
            for r in range(12):
                psw = ps2_pool.tile([P, 512], f32, tag="ps2")
                nc.tensor.matmul(psw[:], wu[:, 0:P], wu[:], start=True,
                                 stop=True)

            # Head: interleave w1 m-blocks with x chunks so GEMM1's m-loop
            # on chunk0 is never starved; small tensors go via the
            # Activation queue to keep the Sync issue stream tight.
            b1_sb = sm_pool.tile([P, M1], f32, tag="b1")
            nc.scalar.dma_start(b1_sb[:], b1[:])
            load_w1(0, 1)
            load_chunk(0)
            load_w1(1, 5)
            if NCH > 1:
                load_chunk(1)
            load_w1(5, M1)
            if NCH > 2:
                load_chunk(2)
            w2a = w_pool.tile([P, K2 * D1 * P], BF16, tag="w2a", name="w2a")
            nc.sync.dma_start(w2a[:, 0:K2 * D1 * P // 2],
                              w2p[:, 0:K2 * D1 * P // 2])
            if NCH > 3:
                load_chunk(3)
            nc.sync.dma_start(w2a[:, K2 * D1 * P // 2:],
                              w2p[:, K2 * D1 * P // 2:])
            for ci in range(4, NCH):
                load_chunk(ci)

            ht = ht_pool.tile([P, M1 * C], BF16, tag="ht", name="ht")

            def g1(ci):
                n0, w = chunks[ci]
                cb = K1 * n0
                hb = M1 * n0
                for m in range(M1):
                    ps = ps1_pool.tile([P, 512], f32, tag="ps1")
                    for k in range(K1):
                        nc.tensor.matmul(
                            ps[:, :w],
                            w1a[:, (m * K1 + k) * P:(m * K1 + k + 1) * P],
                            xt[:, cb + k * w:cb + (k + 1) * w],
                            start=(k == 0),
                            stop=(k == K1 - 1),
                        )
                    nc.scalar.activation(
                        ht[:, hb + m * w:hb + (m + 1) * w], ps[:, :w],
                        mybir.ActivationFunctionType.Relu,
                        bias=b1_sb[:, m:m + 1],
                    )

            def g2(ci):
                n0, w = chunks[ci]
                hb = M1 * n0
                last = ci == NCH - 1
                yt = yo_pool.tile([P, D1 * 512], BF16, tag="yo")
                for d in range(D1):
                    ps = ps2_pool.tile([P, 512], f32, tag="ps2")
                    for k in range(K2):
                        nc.tensor.matmul(
                            ps[:, :w],
                            w2a[:, (k * D1 + d) * P:(k * D1 + d + 1) * P],
                            ht[:, hb + k * w:hb + (k + 1) * w],
                            start=(k == 0),
                            stop=(k == K2 - 1),
                        )
                    # psum -> sbuf cast on the otherwise-idle Vector engine
                    nc.vector.tensor_copy(yt[:, d * w:(d + 1) * w], ps[:, :w])
                # One contiguous DMA per chunk; the last chunk's goes via
                # the idle Sync queue so it isn't stuck behind Scalar.
                eng = nc.sync if last else nc.scalar
                eng.dma_start(y[:, D1 * n0:D1 * (n0 + w)], yt[:, 0:D1 * w])

            # Software pipeline: GEMM2 of chunk i runs one chunk behind
            # GEMM1 so the ReLU activations have time to drain.
            g1(0)
            for ci in range(1, NCH):
                g1(ci)
                g2(ci - 1)
            g2(NCH - 1)

    nc.compile()
    return nc


def _pack_chunk_major(mat, chunks, kb):
    """[C, kb*P] row-major -> [P, kb*C] chunk-major ([p][chunk][k][tok])."""
    segs = []
    for n0, w in chunks:
        seg = mat[n0:n0 + w].T.reshape(kb, P, w).transpose(1, 0, 2)
        segs.append(seg.reshape(P, kb * w))
    return np.ascontiguousarray(np.concatenate(segs, axis=1))


def kernel(inputs, embed, router_weights, w1_weight, w1_bias, w2_weight,
           w2_bias, mask):
    inputs = np.asarray(inputs, np.float32)
    embed = np.asarray(embed, np.float32)
    router_weights = np.asarray(router_weights, np.float32)
    w1_weight = np.asarray(w1_weight, np.float32)
    w1_bias = np.asarray(w1_bias, np.float32)
    w2_weight = np.asarray(w2_weight, np.float32)
    w2_bias = np.asarray(w2_bias, np.float32)
    mask_f = np.asarray(mask).astype(np.float32)

    B, T, D = inputs.shape
    N = B * T
    x = inputs.reshape(N, D)

    # ---- host router: softmax top-1 over concat(embed, inputs) ----
    router_in = np.concatenate([embed.reshape(N, EMBED_DIM), x], axis=1)
    logits = router_in @ router_weights
    logits -= logits.max(axis=1, keepdims=True)
    p = np.exp(logits)
    p /= p.sum(axis=1, keepdims=True)
    gate_idx = np.argmax(p, axis=1)
    gate_val = p[np.arange(N), gate_idx] * mask_f.reshape(N)

    # ---- dispatch: expert e -> cores 2e, 2e+1 ----
    shard_idx = []
    for e in range(NUM_EXPERTS):
        te = np.nonzero(gate_idx == e)[0]
        h = (len(te) + 1) // 2
        shard_idx.append(te[:h])
        shard_idx.append(te[h:])
    C = max(32, -(-max(len(s) for s in shard_idx) // 32) * 32)

    nc = _build_nc(C)
    chunks = _chunks_for(C)

    in_maps = []
    for c in range(N_CORES):
        e = c // 2
        idx = shard_idx[c]
        xs = np.zeros((C, D), np.float32)
        xs[: len(idx)] = x[idx]
        xTp = _pack_chunk_major(xs, chunks, K1).astype(NPBF16)
        # w1p [P, M1*K1*P]: [p, m, k, j] = W1T[k*128+p, m*128+j]
        w1p = np.ascontiguousarray(
            w1_weight[e].T.reshape(K1, P, M1, P)
            .transpose(1, 2, 0, 3).reshape(P, M1 * K1 * P)
        ).astype(NPBF16)
        # w2p [P, K2*D1*P]: [p, k, d, j] = W2T[k*128+p, d*128+j]
        w2p = np.ascontiguousarray(
            w2_weight[e].T.reshape(K2, P, D1, P)
            .transpose(1, 0, 2, 3).reshape(P, K2 * D1 * P)
        ).astype(NPBF16)
        b1p = np.ascontiguousarray(w1_bias[e].reshape(M1, P).T)
        in_maps.append({"xT": xTp, "w1p": w1p, "w2p": w2p, "b1": b1p})

    trace = bool(os.environ.get("KERNEL_TRACE"))
    kw = {}
    if trace:
        bass_utils.upload_artifacts = lambda tmpdir: f"local:{tmpdir}"
        kw = dict(trace=True, trace_cores=list(range(N_CORES)),
                  tmpdir=os.environ.get("KERNEL_TRACE_DIR") or None)
    try:
        res = bass_utils.run_bass_kernel_spmd(
            nc, in_maps, core_ids=list(range(N_CORES)), **kw)
    except Exception:
        res = bass_utils.run_bass_kernel_spmd(
            nc, in_maps, core_ids=list(range(N_CORES)), **kw)
    if trace:
        kernel.exec_time_ns = res.exec_time_ns
        kernel.mean_exec_time_ns = res.mean_exec_time_ns

    out = np.zeros((N, D), np.float32)
    for c in range(N_CORES):
        idx = shard_idx[c]
        arr = np.asarray(res.results[c]["y"]).astype(np.float32)
        rows = np.empty((C, D), np.float32)
        for n0, w in chunks:
            seg = arr[:, D1 * n0:D1 * (n0 + w)].reshape(P, D1, w)
            rows[n0:n0 + w] = seg.transpose(2, 1, 0).reshape(w, D1 * P)
        out[idx] = rows[: len(idx)]
    if np.any(w2_bias):
        out += w2_bias[gate_idx]
    out *= gate_val[:, None]
    return out.reshape(B, T, D)
